# revision 1
# baseline (speedup 1.0000x reference)
"""Cross-attention layer (B=2, T=2048, C=3072, D=1024, 16 heads x 64) on 8 TRN2 cores.

Sharding: batch (2) x head-groups (4). Core i handles batch b=i//4 and the 4
heads [4*(i%4), 4*(i%4)+4). Q/K/V projections are column-sharded by head,
the output projection is row-sharded, so each core returns a partial [T, D]
output; the host sums the 4 partials per batch element and adds bo.

On-core dataflow (all matmuls in float32r = full PE rate, fp32 PSUM accum):
  qT[256,2048] = WqT_shard^T-chunks @ xT        (transposed layouts)
  kT[256,3072], vT -> PE-transpose -> v[c,260]  (65 cols/head: 64 v + ones col)
  per head: sT[c-tile, t] = kT_h^T-slice @ qT_h ;  exp on ACT (scale=1/8 fused)
            uo_ps[65, t] += [v_h | 1]^T @ exp   (row 64 = softmax denominator)
  normalize: denominators broadcast via mask-matmul, reciprocal+mul on DVE
  out[t, D] = sum_h uo_h^T-slice @ WoT_shard    (heads accumulated in PSUM)
"""
import os

import numpy as np
import concourse.bass as bass  # noqa: F401  (engine types re-exported via nc)
import concourse.mybir as mybir
import concourse.tile as tile
from concourse import bacc
from concourse.bass import ds, ts
from concourse.bass_utils import run_bass_kernel_spmd
import ml_dtypes

_bf16 = ml_dtypes.bfloat16

B, T, C, D = 2, 2048, 3072, 1024
NH, DH = 16, 64
NCORES = 8
HL = 4                # heads per core
DL = HL * DH          # 256 local projection dims
DHP = DH + 1          # 65: per-head v columns incl. ones column

F32 = mybir.dt.float32
F32R = mybir.dt.float32r
BF16 = mybir.dt.bfloat16
AF = mybir.ActivationFunctionType

KC = D // 128         # 8 contraction chunks for projections
CC = C // 128         # 24 key tiles
TT = T // 128         # 16 query tiles


DBG_PHASES = os.environ.get("KERNEL_PHASES", "ABC")
DBG_SUB = int(os.environ.get("KERNEL_SUB", "3"))  # 1=Q only, 2=+K, 3=+V


def _emit(nc, tc, io):
    xT, ctxT, wqkvT, woT, bqkv, msk, iden, ones, out = io

    with tc.sbuf_pool(name="persist", bufs=1) as pp:
        qT = [pp.tile([128, T], BF16, name=f"qT{p}") for p in range(2)]
        kT = [pp.tile([128, C], BF16, name=f"kT{p}") for p in range(2)]
        v = [pp.tile([128, HL * DHP], BF16, name=f"v{ci}") for ci in range(CC)]
        wo = [pp.tile([128, D], F32R, name=f"wo{p}") for p in range(2)]
        msk_sb = pp.tile([65, 128], F32, name="msk_sb")
        iden_sb = pp.tile([128, 128], F32, name="iden_sb")
        b_sb = [pp.tile([128, 3], F32, name=f"b{p}") for p in range(2)]
        for p in range(2):
            nc.sync.dma_start(out=wo[p], in_=woT[ts(p, 128), :].bitcast(F32R))
            nc.sync.dma_start(out=b_sb[p], in_=bqkv[ts(p, 128), :])
        bq_sb = [b_sb[p][:, 0:1] for p in range(2)]
        bk_sb = [b_sb[p][:, 1:2] for p in range(2)]
        bv_sb = [b_sb[p][:, 2:3] for p in range(2)]
        nc.sync.dma_start(out=msk_sb, in_=msk)
        nc.sync.dma_start(out=iden_sb, in_=iden)
        for ci in range(CC):
            # ones column per head (position DH of each 65-col head block)
            dst = v[ci].rearrange("a (h c) -> a h c", h=HL)[:, :, DH:DHP]
            nc.sync.dma_start(out=dst, in_=ones[:, :, None])

        # ---------------- Phase A: projections ----------------
        with (
            tc.sbuf_pool(name="wqkv", bufs=1) as wp,
            tc.sbuf_pool(name="stream", bufs=1) as sp,
            tc.sbuf_pool(name="vstage", bufs=4) as vp,
            tc.psum_pool(name="pps", bufs=4) as pps,
            tc.psum_pool(name="tps", bufs=2) as tps,
        ):
            w_sb = [wp.tile([128, 3 * DL], BF16, name=f"w{k}") for k in range(KC)]
            xs = [sp.tile([128, T], BF16, tag=f"x{k}", name=f"xs{k}") for k in range(KC)]
            cs = [sp.tile([128, C], BF16, tag=f"c{k}", name=f"cs{k}") for k in range(KC)]
            for k in range(KC):
                nc.sync.dma_start(out=w_sb[k], in_=wqkvT[ts(k, 128), :])
                nc.sync.dma_start(out=xs[k], in_=xT[ts(k, 128), :])
                nc.sync.dma_start(out=cs[k], in_=ctxT[ts(k, 128), :])
            wq = [w_sb[k][:, 0:DL] for k in range(KC)]
            wk = [w_sb[k][:, DL : 2 * DL] for k in range(KC)]
            wv = [w_sb[k][:, 2 * DL : 3 * DL] for k in range(KC)]

            # Q projection
            for p in range(2):
                for tq in range(4):
                    ps = pps.tile([128, 512], F32, tag="proj", name=f"qp{p}{tq}")
                    for k in range(KC):
                        nc.tensor.matmul(
                            ps,
                            lhsT=wq[k][:, ts(p, 128)],
                            rhs=xs[k][:, ts(tq, 512)],
                            start=(k == 0),
                            stop=(k == KC - 1),
                        )
                    nc.vector.tensor_scalar_add(
                        out=qT[p][:, ts(tq, 512)], in0=ps, scalar1=bq_sb[p]
                    )

            # V projection + transposes first, K projection last: the phase-A
            # tail is then pure dense PE matmul work whose early chunks already
            # unblock attention, so the PE rolls into QK with no idle gap
            # (a >3.4us PE gap here latches the HAM throttle for the whole
            # ACT-paced attention phase).
            for p in range(2):
                for cq in range(6):
                    ps2 = pps.tile([128, 512], F32, tag="proj", name=f"vp{p}{cq}")
                    for k in range(KC):
                        nc.tensor.matmul(
                            ps2,
                            lhsT=wv[k][:, ts(p, 128)],
                            rhs=cs[k][:, ts(cq, 512)],
                            start=(k == 0),
                            stop=(k == KC - 1),
                        )
                    vst = vp.tile([128, 512], F32, tag="vst", name=f"vs{p}{cq}")
                    nc.vector.tensor_scalar_add(out=vst, in0=ps2, scalar1=bv_sb[p])
                    for cb in range(4):
                        ci = cq * 4 + cb
                        tp_ = tps.tile([128, 128], F32, tag="tr", name=f"tr{ci}{p}")
                        nc.tensor.transpose(tp_, vst[:, ts(cb, 128)], iden_sb)
                        src_ = tp_.rearrange("a (h c) -> a h c", h=2)
                        dst = v[ci][:, ds(2 * p * DHP, 2 * DHP)].rearrange(
                            "a (h c) -> a h c", h=2
                        )[:, :, 0:DH]
                        nc.vector.tensor_copy(out=dst, in_=src_)

            for p in range(2):
                for cq in range(6):
                    ps = pps.tile([128, 512], F32, tag="proj", name=f"kp{p}{cq}")
                    for k in range(KC):
                        nc.tensor.matmul(
                            ps,
                            lhsT=wk[k][:, ts(p, 128)],
                            rhs=cs[k][:, ts(cq, 512)],
                            start=(k == 0),
                            stop=(k == KC - 1),
                        )
                    nc.vector.tensor_scalar_add(
                        out=kT[p][:, ts(cq, 512)], in0=ps, scalar1=bk_sb[p]
                    )

        if "B" not in DBG_PHASES:
            # dump qT/kT so the kernel has output and deps
            with tc.sbuf_pool(name="dbg", bufs=2) as db:
                for p in range(2):
                    dq = db.tile([128, T], F32, tag="dq", name=f"dq{p}")
                    nc.vector.tensor_copy(out=dq, in_=qT[p].bitcast(F32))
                    nc.sync.dma_start(out=out[ts(2 * p, 128), 0:1024], in_=dq[:, 0:1024])
                    nc.sync.dma_start(out=out[ts(2 * p + 1, 128), 0:1024], in_=dq[:, 1024:2048])
            return

        # ---------------- Phases B + C ----------------
        with (
            tc.sbuf_pool(name="uop", bufs=1) as up,
            tc.sbuf_pool(name="rsp", bufs=1) as rp,
            tc.sbuf_pool(name="obp", bufs=3) as ob,
            tc.sbuf_pool(name="expp", bufs=3) as ep,
            tc.sbuf_pool(name="rcp", bufs=2) as rc,
            tc.psum_pool(name="qkps", bufs=2) as qps,
            tc.psum_pool(name="uops", bufs=2) as ups,
        ):
            uo = [up.tile([128, T], F32R, name=f"uo{pr}") for pr in range(2)]
            rs = [rp.tile([65, T], F32, name=f"rs{pr}") for pr in range(2)]
            for pr in range(2):
                nc.gpsimd.memset(rs[pr], 1.0)

            def norm_pair(pr, th):
                # broadcast denominators [2 rows] -> [128, 1024], recip, scale uo
                bc = qps.tile([128, 1024], F32, tag="qk", name=f"bc{pr}_{th}")
                for tq in range(2):
                    nc.tensor.matmul(
                        bc[:, ts(tq, 512)],
                        lhsT=msk_sb,
                        rhs=rs[pr][:, ds(th * 1024 + tq * 512, 512)],
                        start=True,
                        stop=True,
                    )
                rcl = rc.tile([128, 1024], F32, tag="rc", name=f"rcl{pr}_{th}")
                nc.vector.reciprocal_approx_fast(rcl, bc)
                nc.vector.tensor_mul(
                    out=uo[pr][:, ds(th * 1024, 1024)],
                    in0=uo[pr][:, ds(th * 1024, 1024)],
                    in1=rcl,
                )

            def attn_head(h):
                p, off = h // 2, 64 * (h % 2)
                qTh = qT[p][ds(off, 64), :]
                kTh = kT[p][ds(off, 64), :]
                for tb in range(2):
                    uo_ps = ups.tile([65, 1024], F32, tag="uo", name=f"up{h}_{tb}")

                    def pv_step(ci, ex):
                        vsl = v[ci][:, ds(h * DHP, DHP)]
                        for tq in range(2):
                            nc.tensor.matmul(
                                uo_ps[:, ts(tq, 512)],
                                lhsT=vsl,
                                rhs=ex[:, ts(tq, 512)],
                                start=(ci == 0),
                                stop=(ci == CC - 1),
                            )

                    pending = None
                    for ci in range(CC):
                        qk = qps.tile(
                            [128, 1024], F32, tag="qk", name=f"qk{h}_{tb}_{ci}"
                        )
                        for tq in range(2):
                            nc.tensor.matmul(
                                qk[:, ts(tq, 512)],
                                lhsT=kTh[:, ts(ci, 128)],
                                rhs=qTh[:, ds(tb * 1024 + tq * 512, 512)],
                                start=True,
                                stop=True,
                            )
                        ex = ep.tile(
                            [128, 1024], BF16, tag="exp", name=f"ex{h}_{tb}_{ci}"
                        )
                        nc.scalar.activation(ex, qk, AF.Exp, scale=0.125)
                        if pending is not None:
                            pv_step(*pending)
                        pending = (ci, ex)
                    pv_step(*pending)
                    nc.vector.tensor_copy(
                        out=uo[p][ds(off, 64), ds(tb * 1024, 1024)],
                        in_=uo_ps[0:64, :],
                    )
                    nc.vector.tensor_copy(
                        out=rs[p][ds(64 * (h % 2), 1), ds(tb * 1024, 1024)],
                        in_=uo_ps[64:65, :],
                    )

            def phase_c_half(th):
                # output projection for t-tiles in half `th`
                for tt in range(th * (TT // 2), (th + 1) * (TT // 2)):
                    osb = ob.tile([128, D], F32, tag="ob", name=f"ob{tt}")
                    for dc in range(2):
                        o_ps = ups.tile([128, 512], F32, tag="uo", name=f"o{tt}_{dc}")
                        for p in range(2):
                            nc.tensor.matmul(
                                o_ps,
                                lhsT=uo[p][:, ts(tt, 128)],
                                rhs=wo[p][:, ts(dc, 512)],
                                start=(p == 0),
                                stop=(p == 1),
                            )
                        nc.vector.tensor_copy(out=osb[:, ts(dc, 512)], in_=o_ps)
                    nc.sync.dma_start(out=out[ts(tt, 128), :], in_=osb)

            for h in range(HL):
                attn_head(h)
            norm_pair(0, 0)
            norm_pair(1, 0)
            phase_c_half(0)
            norm_pair(0, 1)
            norm_pair(1, 1)
            phase_c_half(1)

def _build_nc():
    nc = bacc.Bacc("TRN2", target_bir_lowering=False, debug=False, num_devices=NCORES)
    xT = nc.dram_tensor("xT", [D, T], BF16, kind="ExternalInput").ap()
    ctxT = nc.dram_tensor("ctxT", [D, C], BF16, kind="ExternalInput").ap()
    wqkvT = nc.dram_tensor("wqkvT", [D, 3 * DL], BF16, kind="ExternalInput").ap()
    woT = nc.dram_tensor("woT", [DL, D], F32, kind="ExternalInput").ap()
    bqkv = nc.dram_tensor("bqkv", [DL, 3], F32, kind="ExternalInput").ap()
    msk = nc.dram_tensor("msk", [65, 128], F32, kind="ExternalInput").ap()
    iden = nc.dram_tensor("iden", [128, 128], F32, kind="ExternalInput").ap()
    ones = nc.dram_tensor("ones", [128, HL], BF16, kind="ExternalInput").ap()
    out = nc.dram_tensor("out", [T, D], F32, kind="ExternalOutput").ap()
    with tile.TileContext(nc) as tc:
        _emit(nc, tc, (xT, ctxT, wqkvT, woT, bqkv, msk, iden, ones, out))
    nc.compile()
    return nc


_NC_CACHE = None


def _get_nc():
    global _NC_CACHE
    if _NC_CACHE is None:
        _NC_CACHE = _build_nc()
    return _NC_CACHE


def _make_in_maps(inputs):
    x = np.asarray(inputs["x"], dtype=np.float32)
    context = np.asarray(inputs["context"], dtype=np.float32)
    Wq = np.asarray(inputs["Wq"], dtype=np.float32)
    Wk = np.asarray(inputs["Wk"], dtype=np.float32)
    Wv = np.asarray(inputs["Wv"], dtype=np.float32)
    Wo = np.asarray(inputs["Wo"], dtype=np.float32)
    bq = np.asarray(inputs["bq"], dtype=np.float32)
    bk = np.asarray(inputs["bk"], dtype=np.float32)
    bv = np.asarray(inputs["bv"], dtype=np.float32)

    msk = np.zeros((65, 128), np.float32)
    msk[0, :64] = 1.0
    msk[64, 64:] = 1.0
    iden = np.eye(128, dtype=np.float32)

    xTs = [np.ascontiguousarray(x[b].T).astype(_bf16) for b in range(B)]
    cTs = [np.ascontiguousarray(context[b].T).astype(_bf16) for b in range(B)]

    in_maps = []
    for core in range(NCORES):
        b, hg = core // 4, core % 4
        sl = slice(hg * DL, (hg + 1) * DL)
        in_maps.append(
            {
                "xT": xTs[b],
                "ctxT": cTs[b],
                "wqkvT": np.ascontiguousarray(
                    np.concatenate([Wq[sl].T, Wk[sl].T, Wv[sl].T], axis=1)
                ).astype(_bf16),
                "woT": np.ascontiguousarray(Wo[:, sl].T),
                "bqkv": np.ascontiguousarray(
                    np.stack([bq[sl], bk[sl], bv[sl]], axis=1)
                ),
                "msk": msk,
                "iden": iden,
                "ones": np.ones((128, HL), _bf16),
            }
        )
    return in_maps


def run_spmd(inputs, trace=False):
    """Run the SPMD kernel; returns (full output [B,T,D], BassKernelResults)."""
    in_maps = _make_in_maps(inputs)
    res = run_bass_kernel_spmd(
        _get_nc(), in_maps, core_ids=list(range(NCORES)), trace=trace
    )
    bo = np.asarray(inputs["bo"], dtype=np.float32)
    y = np.zeros((B, T, D), np.float32)
    for core in range(NCORES):
        y[core // 4] += res.results[core]["out"]
    y += bo.reshape(1, 1, D)
    return y, res


def kernel(**inputs):
    y, _ = run_spmd(inputs, trace=False)
    return y



# revision 4
# speedup vs baseline: 1.5256x; 1.5256x over previous
"""Cross-attention layer (B=2, T=2048, C=3072, D=1024, 16 heads x 64) on 8 TRN2 cores.

Sharding: batch (2) x head-groups (4). Core i handles batch b=i//4 and the 4
heads [4*(i%4), 4*(i%4)+4). Q/K/V projections are column-sharded by head,
the output projection is row-sharded, so each core returns a partial [T, D]
output (bf16); the host sums the 4 partials per batch element and adds bo.

On-core dataflow (bf16 matmuls, fp32 PSUM accum):
  qT[256,2048] = WqT_shard^T-chunks @ xT        (transposed layouts)
  kT[256,3072], vT -> PE-transpose -> v[c,260]  (65 cols/head: 64 v + ones col)
  attention runs per head-PAIR (2p, 2p+1): the two heads' QK matmuls are
  issued adjacently into PE row-tiles (0,0)/(64,0) so they execute
  concurrently in the 128x128 array (full-array activity keeps the HAM
  clock-gate warm); exp on ACT (scale=1/8 fused); PV accumulates
  uo_ps[65, t] per head (row 64 = softmax denominator via the ones column).
  normalize: denominators broadcast via mask-matmul, reciprocal+mul on DVE
  out[t, D] = sum_p uo_p^T-slice @ WoT_shard    (bf16, accumulated in PSUM)
"""
from collections import deque

import numpy as np
import concourse.bass as bass  # noqa: F401  (engine types re-exported via nc)
import concourse.mybir as mybir
import concourse.tile as tile
from concourse import bacc
from concourse.bass import ds, ts
from concourse.bass_utils import run_bass_kernel_spmd
import ml_dtypes

_bf16 = ml_dtypes.bfloat16

B, T, C, D = 2, 2048, 3072, 1024
NH, DH = 16, 64
NCORES = 8
HL = 4                # heads per core
DL = HL * DH          # 256 local projection dims
DHP = DH + 1          # 65: per-head v columns incl. ones column

F32 = mybir.dt.float32
BF16 = mybir.dt.bfloat16
AF = mybir.ActivationFunctionType

KC = D // 128         # 8 contraction chunks for projections
CC = C // 128         # 24 key tiles
TT = T // 128         # 16 query tiles


def _emit(nc, tc, io):
    xT, ctxT, wqkvT, woT, bqkv, msk, iden, out = io

    with tc.sbuf_pool(name="persist", bufs=1) as pp:
        qT = [pp.tile([128, T], BF16, name=f"qT{p}") for p in range(2)]
        kT = [pp.tile([128, C], BF16, name=f"kT{p}") for p in range(2)]
        v = pp.tile([128, CC * HL * DHP], BF16, name="v")
        wo = [pp.tile([128, D], BF16, name=f"wo{p}") for p in range(2)]
        msk_sb = pp.tile([65, 128], BF16, name="msk_sb")
        iden_sb = pp.tile([128, 128], BF16, name="iden_sb")
        b_sb = [pp.tile([128, 3], F32, name=f"b{p}") for p in range(2)]
        # ones columns for the softmax denominators: engine memset, not DMA
        # (24 strided 2-byte-packet DMAs at the head of the input queue cost
        # ~40us of dead PE time in the previous revision).
        ones_view = v.rearrange("a (i c) -> a i c", c=DHP)[:, :, DH:DHP]
        nc.gpsimd.memset(ones_view, 1.0)

        for p in range(2):
            nc.sync.dma_start(out=b_sb[p], in_=bqkv[ts(p, 128), :])
        bq_sb = [b_sb[p][:, 0:1] for p in range(2)]
        bk_sb = [b_sb[p][:, 1:2] for p in range(2)]
        bv_sb = [b_sb[p][:, 2:3] for p in range(2)]
        nc.sync.dma_start(out=iden_sb, in_=iden)

        # ---------------- Phase A: projections ----------------
        with (
            tc.sbuf_pool(name="wqkv", bufs=1) as wp,
            tc.sbuf_pool(name="stream", bufs=1) as sp,
            tc.sbuf_pool(name="vstage", bufs=4) as vp,
            tc.psum_pool(name="pps", bufs=4) as pps,
            tc.psum_pool(name="tps", bufs=2) as tps,
        ):
            w_sb = [wp.tile([128, 3 * DL], BF16, name=f"w{k}") for k in range(KC)]
            xs = [sp.tile([128, T], BF16, tag=f"x{k}", name=f"xs{k}") for k in range(KC)]
            cs = [sp.tile([128, C], BF16, tag=f"c{k}", name=f"cs{k}") for k in range(KC)]
            # DMA priority order on the (single) dynamic queue: weights and x
            # first (unblocks Q-projection ~6us in), then context, then the
            # late-phase constants (msk, wo).
            for k in range(KC):
                nc.sync.dma_start(out=w_sb[k], in_=wqkvT[ts(k, 128), :])
            for k in range(KC):
                nc.sync.dma_start(out=xs[k], in_=xT[ts(k, 128), :])
            for k in range(KC):
                nc.sync.dma_start(out=cs[k], in_=ctxT[ts(k, 128), :])
            nc.sync.dma_start(out=msk_sb, in_=msk)
            for p in range(2):
                nc.sync.dma_start(out=wo[p], in_=woT[ts(p, 128), :])

            wq = [w_sb[k][:, 0:DL] for k in range(KC)]
            wk = [w_sb[k][:, DL : 2 * DL] for k in range(KC)]
            wv = [w_sb[k][:, 2 * DL : 3 * DL] for k in range(KC)]

            # Q projection
            for p in range(2):
                for tq in range(4):
                    ps = pps.tile([128, 512], F32, tag="proj", name=f"qp{p}{tq}")
                    for k in range(KC):
                        nc.tensor.matmul(
                            ps,
                            lhsT=wq[k][:, ts(p, 128)],
                            rhs=xs[k][:, ts(tq, 512)],
                            start=(k == 0),
                            stop=(k == KC - 1),
                        )
                    nc.vector.tensor_scalar_add(
                        out=qT[p][:, ts(tq, 512)], in0=ps, scalar1=bq_sb[p]
                    )

            # V projection + transposes first, K projection last: the phase-A
            # tail is then pure dense PE matmul work.
            for p in range(2):
                for cq in range(6):
                    ps2 = pps.tile([128, 512], F32, tag="proj", name=f"vp{p}{cq}")
                    for k in range(KC):
                        nc.tensor.matmul(
                            ps2,
                            lhsT=wv[k][:, ts(p, 128)],
                            rhs=cs[k][:, ts(cq, 512)],
                            start=(k == 0),
                            stop=(k == KC - 1),
                        )
                    vst = vp.tile([128, 512], BF16, tag="vst", name=f"vs{p}{cq}")
                    nc.vector.tensor_scalar_add(out=vst, in0=ps2, scalar1=bv_sb[p])
                    for cb in range(4):
                        ci = cq * 4 + cb
                        tp_ = tps.tile([128, 128], BF16, tag="tr", name=f"tr{ci}{p}")
                        nc.tensor.transpose(tp_, vst[:, ts(cb, 128)], iden_sb)
                        src_ = tp_.rearrange("a (h c) -> a h c", h=2)
                        dst = v[:, ds(ci * HL * DHP + 2 * p * DHP, 2 * DHP)].rearrange(
                            "a (h c) -> a h c", h=2
                        )[:, :, 0:DH]
                        nc.vector.tensor_copy(out=dst, in_=src_)

            for p in range(2):
                for cq in range(6):
                    ps = pps.tile([128, 512], F32, tag="proj", name=f"kp{p}{cq}")
                    for k in range(KC):
                        nc.tensor.matmul(
                            ps,
                            lhsT=wk[k][:, ts(p, 128)],
                            rhs=cs[k][:, ts(cq, 512)],
                            start=(k == 0),
                            stop=(k == KC - 1),
                        )
                    nc.vector.tensor_scalar_add(
                        out=kT[p][:, ts(cq, 512)], in0=ps, scalar1=bk_sb[p]
                    )

        # ---------------- Phases B + C ----------------
        with (
            tc.sbuf_pool(name="uop", bufs=1) as up,
            tc.sbuf_pool(name="rsp", bufs=1) as rp,
            tc.sbuf_pool(name="obp", bufs=3) as ob,
            tc.sbuf_pool(name="expp", bufs=3) as ep,
            tc.sbuf_pool(name="rcp", bufs=2) as rc,
            tc.psum_pool(name="qkps", bufs=2) as qps,
            tc.psum_pool(name="uops", bufs=2) as ups,
        ):
            uo = [up.tile([128, T], BF16, name=f"uo{pr}") for pr in range(2)]
            rs = [rp.tile([65, T], BF16, name=f"rs{pr}") for pr in range(2)]
            for pr in range(2):
                nc.gpsimd.memset(rs[pr], 1.0)

            def norm_pair(pr, th):
                # broadcast denominators [2 rows] -> [128, 1024], recip, scale uo
                bc = qps.tile([128, 1024], F32, tag="qk", name=f"bc{pr}_{th}")
                for tq in range(2):
                    nc.tensor.matmul(
                        bc[:, ts(tq, 512)],
                        lhsT=msk_sb,
                        rhs=rs[pr][:, ds(th * 1024 + tq * 512, 512)],
                        start=True,
                        stop=True,
                    )
                rcl = rc.tile([128, 1024], F32, tag="rc", name=f"rcl{pr}_{th}")
                nc.vector.reciprocal_approx_fast(rcl, bc)
                nc.vector.tensor_mul(
                    out=uo[pr][:, ds(th * 1024, 1024)],
                    in0=uo[pr][:, ds(th * 1024, 1024)],
                    in1=rcl,
                )

            def attn_pair(p, tb):
                # heads (2p, 2p+1) together: their QK matmuls go to PE row
                # tiles (0,0)/(64,0) back-to-back, executing concurrently.
                uo_ps = [
                    ups.tile([65, 1024], F32, tag="uo", name=f"up{p}{tb}_{h2}")
                    for h2 in range(2)
                ]

                def pv_step(ci, h2, ex):
                    h = 2 * p + h2
                    vsl = v[:, ds((ci * HL + h) * DHP, DHP)]
                    for tq in range(2):
                        nc.tensor.matmul(
                            uo_ps[h2][:, ts(tq, 512)],
                            lhsT=vsl,
                            rhs=ex[:, ts(tq, 512)],
                            start=(ci == 0),
                            stop=(ci == CC - 1),
                        )

                pend = deque()
                for ci in range(CC):
                    exs = []
                    for h2 in range(2):
                        base = 64 * h2
                        qk = qps.tile(
                            [128, 1024], F32, tag="qk", name=f"qk{p}{tb}_{ci}_{h2}"
                        )
                        for tq in range(2):
                            nc.tensor.matmul(
                                qk[:, ts(tq, 512)],
                                lhsT=kT[p][ds(base, 64), ts(ci, 128)],
                                rhs=qT[p][ds(base, 64), ds(tb * 1024 + tq * 512, 512)],
                                start=True,
                                stop=True,
                                tile_position=(base, 0),
                            )
                        ex = ep.tile(
                            [128, 1024], BF16, tag="exp", name=f"ex{p}{tb}_{ci}_{h2}"
                        )
                        nc.scalar.activation(ex, qk, AF.Exp, scale=0.125)
                        exs.append(ex)
                    while pend:
                        pv_step(*pend.popleft())
                    for h2 in range(2):
                        pend.append((ci, h2, exs[h2]))
                while pend:
                    pv_step(*pend.popleft())
                for h2 in range(2):
                    off = 64 * h2
                    nc.vector.tensor_copy(
                        out=uo[p][ds(off, 64), ds(tb * 1024, 1024)],
                        in_=uo_ps[h2][0:64, :],
                    )
                    nc.vector.tensor_copy(
                        out=rs[p][ds(off, 1), ds(tb * 1024, 1024)],
                        in_=uo_ps[h2][64:65, :],
                    )

            def phase_c_half(th):
                # output projection for t-tiles in half `th`
                for tt in range(th * (TT // 2), (th + 1) * (TT // 2)):
                    osb = ob.tile([128, D], BF16, tag="ob", name=f"ob{tt}")
                    for dc in range(2):
                        o_ps = ups.tile([128, 512], F32, tag="uo", name=f"o{tt}_{dc}")
                        for p in range(2):
                            nc.tensor.matmul(
                                o_ps,
                                lhsT=uo[p][:, ts(tt, 128)],
                                rhs=wo[p][:, ts(dc, 512)],
                                start=(p == 0),
                                stop=(p == 1),
                            )
                        nc.vector.tensor_copy(out=osb[:, ts(dc, 512)], in_=o_ps)
                    nc.sync.dma_start(out=out[ts(tt, 128), :], in_=osb)

            for p in range(2):
                for tb in range(2):
                    attn_pair(p, tb)
            norm_pair(0, 0)
            norm_pair(1, 0)
            phase_c_half(0)
            norm_pair(0, 1)
            norm_pair(1, 1)
            phase_c_half(1)


def _build_nc():
    nc = bacc.Bacc("TRN2", target_bir_lowering=False, debug=False, num_devices=NCORES)
    xT = nc.dram_tensor("xT", [D, T], BF16, kind="ExternalInput").ap()
    ctxT = nc.dram_tensor("ctxT", [D, C], BF16, kind="ExternalInput").ap()
    wqkvT = nc.dram_tensor("wqkvT", [D, 3 * DL], BF16, kind="ExternalInput").ap()
    woT = nc.dram_tensor("woT", [DL, D], BF16, kind="ExternalInput").ap()
    bqkv = nc.dram_tensor("bqkv", [DL, 3], F32, kind="ExternalInput").ap()
    msk = nc.dram_tensor("msk", [65, 128], BF16, kind="ExternalInput").ap()
    iden = nc.dram_tensor("iden", [128, 128], BF16, kind="ExternalInput").ap()
    out = nc.dram_tensor("out", [T, D], BF16, kind="ExternalOutput").ap()
    with tile.TileContext(nc) as tc:
        _emit(nc, tc, (xT, ctxT, wqkvT, woT, bqkv, msk, iden, out))
    nc.compile()
    return nc


_NC_CACHE = None


def _get_nc():
    global _NC_CACHE
    if _NC_CACHE is None:
        _NC_CACHE = _build_nc()
    return _NC_CACHE


def _make_in_maps(inputs):
    x = np.asarray(inputs["x"], dtype=np.float32)
    context = np.asarray(inputs["context"], dtype=np.float32)
    Wq = np.asarray(inputs["Wq"], dtype=np.float32)
    Wk = np.asarray(inputs["Wk"], dtype=np.float32)
    Wv = np.asarray(inputs["Wv"], dtype=np.float32)
    Wo = np.asarray(inputs["Wo"], dtype=np.float32)
    bq = np.asarray(inputs["bq"], dtype=np.float32)
    bk = np.asarray(inputs["bk"], dtype=np.float32)
    bv = np.asarray(inputs["bv"], dtype=np.float32)

    msk = np.zeros((65, 128), _bf16)
    msk[0, :64] = 1.0
    msk[64, 64:] = 1.0
    iden = np.eye(128, dtype=_bf16)

    xTs = [np.ascontiguousarray(x[b].T).astype(_bf16) for b in range(B)]
    cTs = [np.ascontiguousarray(context[b].T).astype(_bf16) for b in range(B)]

    in_maps = []
    for core in range(NCORES):
        b, hg = core // 4, core % 4
        sl = slice(hg * DL, (hg + 1) * DL)
        in_maps.append(
            {
                "xT": xTs[b],
                "ctxT": cTs[b],
                "wqkvT": np.ascontiguousarray(
                    np.concatenate([Wq[sl].T, Wk[sl].T, Wv[sl].T], axis=1)
                ).astype(_bf16),
                "woT": np.ascontiguousarray(Wo[:, sl].T).astype(_bf16),
                "bqkv": np.ascontiguousarray(
                    np.stack([bq[sl], bk[sl], bv[sl]], axis=1)
                ),
                "msk": msk,
                "iden": iden,
            }
        )
    return in_maps


def run_spmd(inputs, trace=False):
    """Run the SPMD kernel; returns (full output [B,T,D], BassKernelResults)."""
    in_maps = _make_in_maps(inputs)
    res = run_bass_kernel_spmd(
        _get_nc(), in_maps, core_ids=list(range(NCORES)), trace=trace
    )
    bo = np.asarray(inputs["bo"], dtype=np.float32)
    y = np.zeros((B, T, D), np.float32)
    for core in range(NCORES):
        y[core // 4] += np.asarray(res.results[core]["out"], dtype=np.float32)
    y += bo.reshape(1, 1, D)
    return y, res


def kernel(**inputs):
    y, _ = run_spmd(inputs, trace=False)
    return y


# revision 6
# speedup vs baseline: 1.7954x; 1.1769x over previous
"""Cross-attention layer (B=2, T=2048, C=3072, D=1024, 16 heads x 64) on 8 TRN2 cores.

Sharding: batch (2) x head-groups (4). Core i handles batch b=i//4 and the 4
heads [4*(i%4), 4*(i%4)+4). Q/K/V projections are column-sharded by head,
the output projection is row-sharded, so each core returns a partial [T, D]
output (bf16); the host sums the 4 partials per batch element and adds bo.

On-core dataflow (bf16 matmuls, fp32 PSUM accum):
  phase A: qT[256,2048], then V -> PE-transpose -> v[c,260] (65 cols/head:
  64 v + ones col from a memset, not DMA).
  attention runs per head-PAIR (2p, 2p+1): the two heads' QK matmuls are
  issued adjacently into PE row-tiles (0,0)/(64,0) so they execute
  concurrently in the 128x128 array; exp on ACT (scale=1/8 fused); PV
  accumulates uo_ps[65, t] per head (row 64 = softmax denominator).
  The K projection (pairs 1-2) and first half of the output projection
  (pairs 3-4) are interleaved INTO the attention ci-loop as just-in-time
  "filler" blocks: their full-128-row matmuls keep the PE HAM clock-gate
  warm (the 64/65-row attention matmuls alone hover below the activity
  threshold and run at half clock), and they hide K/C-phase latency under
  the ACT-bound softmax stream.
  normalize: denominators broadcast via mask-matmul, reciprocal+mul on DVE
  out[t, D] = sum_p uo_p^T-slice @ WoT_shard    (bf16, accumulated in PSUM)
"""
from collections import deque

import numpy as np
import concourse.bass as bass  # noqa: F401  (engine types re-exported via nc)
import concourse.mybir as mybir
import concourse.tile as tile
from concourse import bacc
from concourse.bass import ds, ts
from concourse.bass_utils import run_bass_kernel_spmd
import ml_dtypes

_bf16 = ml_dtypes.bfloat16

B, T, C, D = 2, 2048, 3072, 1024
NH, DH = 16, 64
NCORES = 8
HL = 4                # heads per core
DL = HL * DH          # 256 local projection dims
DHP = DH + 1          # 65: per-head v columns incl. ones column

F32 = mybir.dt.float32
BF16 = mybir.dt.bfloat16
AF = mybir.ActivationFunctionType

KC = D // 128         # 8 contraction chunks for projections
CC = C // 128         # 24 key tiles
TT = T // 128         # 16 query tiles


def _emit(nc, tc, io):
    xT, ctxT, wqkvT, woT, bqkv, msk, iden, out = io

    with (
        tc.sbuf_pool(name="persist", bufs=1) as pp,
        tc.sbuf_pool(name="wqkv", bufs=1) as wp,
        tc.sbuf_pool(name="stream", bufs=1) as sp,
    ):
        qT = [pp.tile([128, T], BF16, name=f"qT{p}") for p in range(2)]
        kT = [pp.tile([128, C], BF16, name=f"kT{p}") for p in range(2)]
        v = pp.tile([128, CC * HL * DHP], BF16, name="v")
        wo = [pp.tile([128, D], BF16, name=f"wo{p}") for p in range(2)]
        msk_sb = pp.tile([65, 128], BF16, name="msk_sb")
        iden_sb = pp.tile([128, 128], BF16, name="iden_sb")
        b_sb = [pp.tile([128, 3], F32, name=f"b{p}") for p in range(2)]
        # ones columns for the softmax denominators: engine memset, not DMA
        # (strided 2-byte-packet DMAs at the head of the input queue cost
        # ~40us of dead PE time).
        ones_view = v.rearrange("a (i c) -> a i c", c=DHP)[:, :, DH:DHP]
        nc.gpsimd.memset(ones_view, 1.0)

        for p in range(2):
            nc.sync.dma_start(out=b_sb[p], in_=bqkv[ts(p, 128), :])
        bq_sb = [b_sb[p][:, 0:1] for p in range(2)]
        bk_sb = [b_sb[p][:, 1:2] for p in range(2)]
        bv_sb = [b_sb[p][:, 2:3] for p in range(2)]
        nc.sync.dma_start(out=iden_sb, in_=iden)

        w_sb = [wp.tile([128, 3 * DL], BF16, name=f"w{k}") for k in range(KC)]
        xs = [sp.tile([128, T], BF16, tag=f"x{k}", name=f"xs{k}") for k in range(KC)]
        cs = [sp.tile([128, C], BF16, tag=f"c{k}", name=f"cs{k}") for k in range(KC)]
        # DMA priority order on the (single) dynamic queue: weights and x
        # first (unblocks Q-projection ~6us in), then context, then the
        # late-phase constants (msk, wo).
        for k in range(KC):
            nc.sync.dma_start(out=w_sb[k], in_=wqkvT[ts(k, 128), :])
        for k in range(KC):
            nc.sync.dma_start(out=xs[k], in_=xT[ts(k, 128), :])
        for k in range(KC):
            nc.sync.dma_start(out=cs[k], in_=ctxT[ts(k, 128), :])
        nc.sync.dma_start(out=msk_sb, in_=msk)
        for p in range(2):
            nc.sync.dma_start(out=wo[p], in_=woT[ts(p, 128), :])

        wq = [w_sb[k][:, 0:DL] for k in range(KC)]
        wk = [w_sb[k][:, DL : 2 * DL] for k in range(KC)]
        wv = [w_sb[k][:, 2 * DL : 3 * DL] for k in range(KC)]

        # ---------------- Phase A: Q and V projections ----------------
        with (
            tc.sbuf_pool(name="vstage", bufs=4) as vp,
            tc.psum_pool(name="pps", bufs=4) as pps,
            tc.psum_pool(name="tps", bufs=2) as tps,
        ):
            # Q projection
            for p in range(2):
                for tq in range(4):
                    ps = pps.tile([128, 512], F32, tag="proj", name=f"qp{p}{tq}")
                    for k in range(KC):
                        nc.tensor.matmul(
                            ps,
                            lhsT=wq[k][:, ts(p, 128)],
                            rhs=xs[k][:, ts(tq, 512)],
                            start=(k == 0),
                            stop=(k == KC - 1),
                        )
                    nc.vector.tensor_scalar_add(
                        out=qT[p][:, ts(tq, 512)], in0=ps, scalar1=bq_sb[p]
                    )

            # V projection + transposes (K is interleaved into attention)
            for p in range(2):
                for cq in range(6):
                    ps2 = pps.tile([128, 512], F32, tag="proj", name=f"vp{p}{cq}")
                    for k in range(KC):
                        nc.tensor.matmul(
                            ps2,
                            lhsT=wv[k][:, ts(p, 128)],
                            rhs=cs[k][:, ts(cq, 512)],
                            start=(k == 0),
                            stop=(k == KC - 1),
                        )
                    vst = vp.tile([128, 512], BF16, tag="vst", name=f"vs{p}{cq}")
                    nc.vector.tensor_scalar_add(out=vst, in0=ps2, scalar1=bv_sb[p])
                    for cb in range(4):
                        ci = cq * 4 + cb
                        tp_ = tps.tile([128, 128], BF16, tag="tr", name=f"tr{ci}{p}")
                        nc.tensor.transpose(tp_, vst[:, ts(cb, 128)], iden_sb)
                        src_ = tp_.rearrange("a (h c) -> a h c", h=2)
                        dst = v[:, ds(ci * HL * DHP + 2 * p * DHP, 2 * DHP)].rearrange(
                            "a (h c) -> a h c", h=2
                        )[:, :, 0:DH]
                        nc.vector.tensor_copy(out=dst, in_=src_)

        # ---------------- Phases B + C (K-proj + out-proj interleaved) ----
        with (
            tc.sbuf_pool(name="uop", bufs=1) as up,
            tc.sbuf_pool(name="rsp", bufs=1) as rp,
            tc.sbuf_pool(name="obp", bufs=3) as ob,
            tc.sbuf_pool(name="expp", bufs=3) as ep,
            tc.sbuf_pool(name="rcp", bufs=2) as rc,
            tc.psum_pool(name="qkps", bufs=2) as qps,
            tc.psum_pool(name="uops", bufs=2) as ups,
        ):
            uo = [up.tile([128, T], BF16, name=f"uo{pr}") for pr in range(2)]
            rs = [rp.tile([65, T], BF16, name=f"rs{pr}") for pr in range(2)]
            for pr in range(2):
                nc.gpsimd.memset(rs[pr], 1.0)

            def kproj_block(pp_, cq):
                def emit():
                    ps = qps.tile([128, 512], F32, tag="qk", name=f"kp{pp_}{cq}")
                    for k in range(KC):
                        nc.tensor.matmul(
                            ps,
                            lhsT=wk[k][:, ts(pp_, 128)],
                            rhs=cs[k][:, ts(cq, 512)],
                            start=(k == 0),
                            stop=(k == KC - 1),
                        )
                    nc.vector.tensor_scalar_add(
                        out=kT[pp_][:, ts(cq, 512)], in0=ps, scalar1=bk_sb[pp_]
                    )

                return emit

            def c_tile(tt, on_act=False):
                # one out-projection t-tile: out[tt*128 : , :] = sum_p uo_p @ wo_p
                def emit():
                    osb = ob.tile([128, D], BF16, tag="ob", name=f"ob{tt}")
                    for dc in range(2):
                        o_ps = qps.tile([128, 512], F32, tag="qk", name=f"o{tt}_{dc}")
                        for p in range(2):
                            nc.tensor.matmul(
                                o_ps,
                                lhsT=uo[p][:, ts(tt, 128)],
                                rhs=wo[p][:, ts(dc, 512)],
                                start=(p == 0),
                                stop=(p == 1),
                            )
                        if on_act and dc == 1:
                            nc.scalar.copy(out=osb[:, ts(dc, 512)], in_=o_ps)
                        else:
                            nc.vector.tensor_copy(out=osb[:, ts(dc, 512)], in_=o_ps)
                    nc.sync.dma_start(out=out[ts(tt, 128), :], in_=osb)

                return emit

            def norm_pair(pr, th):
                # broadcast denominators [2 rows] -> [128, 1024], recip, scale uo
                bc = qps.tile([128, 1024], F32, tag="qk", name=f"bc{pr}_{th}")
                for tq in range(2):
                    nc.tensor.matmul(
                        bc[:, ts(tq, 512)],
                        lhsT=msk_sb,
                        rhs=rs[pr][:, ds(th * 1024 + tq * 512, 512)],
                        start=True,
                        stop=True,
                    )
                rcl = rc.tile([128, 1024], F32, tag="rc", name=f"rcl{pr}_{th}")
                nc.vector.reciprocal_approx_fast(rcl, bc)
                nc.vector.tensor_mul(
                    out=uo[pr][:, ds(th * 1024, 1024)],
                    in0=uo[pr][:, ds(th * 1024, 1024)],
                    in1=rcl,
                )

            def attn_pair(p, tb, fillers):
                # heads (2p, 2p+1) together: their QK matmuls go to PE row
                # tiles (0,0)/(64,0) back-to-back, executing concurrently.
                uo_ps = [
                    ups.tile([65, 1024], F32, tag="uo", name=f"up{p}{tb}_{h2}")
                    for h2 in range(2)
                ]

                def pv_step(ci, h2, ex):
                    h = 2 * p + h2
                    vsl = v[:, ds((ci * HL + h) * DHP, DHP)]
                    for tq in range(2):
                        nc.tensor.matmul(
                            uo_ps[h2][:, ts(tq, 512)],
                            lhsT=vsl,
                            rhs=ex[:, ts(tq, 512)],
                            start=(ci == 0),
                            stop=(ci == CC - 1),
                        )

                pend = deque()
                for ci in range(CC):
                    for f in fillers.get(ci, ()):
                        f()
                    exs = []
                    for h2 in range(2):
                        base = 64 * h2
                        qk = qps.tile(
                            [128, 1024], F32, tag="qk", name=f"qk{p}{tb}_{ci}_{h2}"
                        )
                        for tq in range(2):
                            nc.tensor.matmul(
                                qk[:, ts(tq, 512)],
                                lhsT=kT[p][ds(base, 64), ts(ci, 128)],
                                rhs=qT[p][ds(base, 64), ds(tb * 1024 + tq * 512, 512)],
                                start=True,
                                stop=True,
                                tile_position=(base, 0),
                            )
                        ex = ep.tile(
                            [128, 1024], BF16, tag="exp", name=f"ex{p}{tb}_{ci}_{h2}"
                        )
                        nc.scalar.activation(ex, qk, AF.Exp, scale=0.125)
                        exs.append(ex)
                    while pend:
                        pv_step(*pend.popleft())
                    for h2 in range(2):
                        pend.append((ci, h2, exs[h2]))
                while pend:
                    pv_step(*pend.popleft())
                for h2 in range(2):
                    off = 64 * h2
                    nc.vector.tensor_copy(
                        out=uo[p][ds(off, 64), ds(tb * 1024, 1024)],
                        in_=uo_ps[h2][0:64, :],
                    )
                    nc.vector.tensor_copy(
                        out=rs[p][ds(off, 1), ds(tb * 1024, 1024)],
                        in_=uo_ps[h2][64:65, :],
                    )

            # pair 1 (p0,tb0): K-proj p0 just-in-time, one cq block ahead.
            attn_pair(
                0,
                0,
                {
                    0: [kproj_block(0, 0), kproj_block(0, 1)],
                    4: [kproj_block(0, 2)],
                    8: [kproj_block(0, 3)],
                    12: [kproj_block(0, 4)],
                    16: [kproj_block(0, 5)],
                },
            )
            norm_pair(0, 0)
            # pair 2 (p1,tb0): K-proj p1 just-in-time.
            attn_pair(
                1,
                0,
                {
                    0: [kproj_block(1, 0), kproj_block(1, 1)],
                    4: [kproj_block(1, 2)],
                    8: [kproj_block(1, 3)],
                    12: [kproj_block(1, 4)],
                    16: [kproj_block(1, 5)],
                },
            )
            norm_pair(1, 0)
            # pairs 3-4: interleave the first half of the output projection
            # (t-tiles 0-7, both uo halves already normalized for th=0).
            attn_pair(0, 1, {3 * i + 2: [c_tile(i)] for i in range(4)})
            norm_pair(0, 1)
            attn_pair(1, 1, {3 * i + 2: [c_tile(4 + i)] for i in range(4)})
            norm_pair(1, 1)
            for tt in range(TT // 2, TT):
                c_tile(tt, on_act=True)()


def _build_nc():
    nc = bacc.Bacc("TRN2", target_bir_lowering=False, debug=False, num_devices=NCORES)
    xT = nc.dram_tensor("xT", [D, T], BF16, kind="ExternalInput").ap()
    ctxT = nc.dram_tensor("ctxT", [D, C], BF16, kind="ExternalInput").ap()
    wqkvT = nc.dram_tensor("wqkvT", [D, 3 * DL], BF16, kind="ExternalInput").ap()
    woT = nc.dram_tensor("woT", [DL, D], BF16, kind="ExternalInput").ap()
    bqkv = nc.dram_tensor("bqkv", [DL, 3], F32, kind="ExternalInput").ap()
    msk = nc.dram_tensor("msk", [65, 128], BF16, kind="ExternalInput").ap()
    iden = nc.dram_tensor("iden", [128, 128], BF16, kind="ExternalInput").ap()
    out = nc.dram_tensor("out", [T, D], BF16, kind="ExternalOutput").ap()
    with tile.TileContext(nc) as tc:
        _emit(nc, tc, (xT, ctxT, wqkvT, woT, bqkv, msk, iden, out))
    nc.compile()
    return nc


_NC_CACHE = None


def _get_nc():
    global _NC_CACHE
    if _NC_CACHE is None:
        _NC_CACHE = _build_nc()
    return _NC_CACHE


def _make_in_maps(inputs):
    x = np.asarray(inputs["x"], dtype=np.float32)
    context = np.asarray(inputs["context"], dtype=np.float32)
    Wq = np.asarray(inputs["Wq"], dtype=np.float32)
    Wk = np.asarray(inputs["Wk"], dtype=np.float32)
    Wv = np.asarray(inputs["Wv"], dtype=np.float32)
    Wo = np.asarray(inputs["Wo"], dtype=np.float32)
    bq = np.asarray(inputs["bq"], dtype=np.float32)
    bk = np.asarray(inputs["bk"], dtype=np.float32)
    bv = np.asarray(inputs["bv"], dtype=np.float32)

    msk = np.zeros((65, 128), _bf16)
    msk[0, :64] = 1.0
    msk[64, 64:] = 1.0
    iden = np.eye(128, dtype=_bf16)

    xTs = [np.ascontiguousarray(x[b].T).astype(_bf16) for b in range(B)]
    cTs = [np.ascontiguousarray(context[b].T).astype(_bf16) for b in range(B)]

    in_maps = []
    for core in range(NCORES):
        b, hg = core // 4, core % 4
        sl = slice(hg * DL, (hg + 1) * DL)
        in_maps.append(
            {
                "xT": xTs[b],
                "ctxT": cTs[b],
                "wqkvT": np.ascontiguousarray(
                    np.concatenate([Wq[sl].T, Wk[sl].T, Wv[sl].T], axis=1)
                ).astype(_bf16),
                "woT": np.ascontiguousarray(Wo[:, sl].T).astype(_bf16),
                "bqkv": np.ascontiguousarray(
                    np.stack([bq[sl], bk[sl], bv[sl]], axis=1)
                ),
                "msk": msk,
                "iden": iden,
            }
        )
    return in_maps


def run_spmd(inputs, trace=False):
    """Run the SPMD kernel; returns (full output [B,T,D], BassKernelResults)."""
    in_maps = _make_in_maps(inputs)
    res = run_bass_kernel_spmd(
        _get_nc(), in_maps, core_ids=list(range(NCORES)), trace=trace
    )
    bo = np.asarray(inputs["bo"], dtype=np.float32)
    y = np.zeros((B, T, D), np.float32)
    for core in range(NCORES):
        y[core // 4] += np.asarray(res.results[core]["out"], dtype=np.float32)
    y += bo.reshape(1, 1, D)
    return y, res


def kernel(**inputs):
    y, _ = run_spmd(inputs, trace=False)
    return y


# revision 9
# speedup vs baseline: 1.8207x; 1.0141x over previous
"""Cross-attention layer (B=2, T=2048, C=3072, D=1024, 16 heads x 64) on 8 TRN2 cores.

Sharding: batch (2) x head-groups (4). Core i handles batch b=i//4 and the 4
heads [4*(i%4), 4*(i%4)+4). Q/K/V projections are column-sharded by head,
the output projection is row-sharded, so each core returns a partial [T, D]
output (bf16); the host sums the 4 partials per batch element and adds bo.

On-core dataflow (bf16 matmuls, fp32 PSUM accum):
  phase A: qT[256,2048], then V -> PE-transpose -> v[c,260] (65 cols/head:
  64 v + ones col from a memset, not DMA).
  attention runs per head-PAIR (2p, 2p+1): the two heads' QK matmuls are
  issued adjacently into PE row-tiles (0,0)/(64,0) so they execute
  concurrently in the 128x128 array; exp on ACT (scale=1/8 fused); PV
  accumulates uo_ps[65, t] per head (row 64 = softmax denominator).
  The K projection (pairs 1-2) and first half of the output projection
  (pairs 3-4) are interleaved INTO the attention ci-loop as just-in-time
  "filler" blocks: their full-128-row matmuls keep the PE HAM clock-gate
  warm (the 64/65-row attention matmuls alone hover below the activity
  threshold and run at half clock), and they hide K/C-phase latency under
  the ACT-bound softmax stream.
  normalize: denominators broadcast via mask-matmul, reciprocal+mul on DVE
  out[t, D] = sum_p uo_p^T-slice @ WoT_shard    (bf16, accumulated in PSUM)
"""
from collections import deque

import numpy as np
import concourse.bass as bass  # noqa: F401  (engine types re-exported via nc)
import concourse.mybir as mybir
import concourse.tile as tile
from concourse import bacc
from concourse.bass import ds, ts
from concourse.bass_utils import run_bass_kernel_spmd
import ml_dtypes

_bf16 = ml_dtypes.bfloat16

B, T, C, D = 2, 2048, 3072, 1024
NH, DH = 16, 64
NCORES = 8
HL = 4                # heads per core
DL = HL * DH          # 256 local projection dims
DHP = DH + 1          # 65: per-head v columns incl. ones column

F32 = mybir.dt.float32
BF16 = mybir.dt.bfloat16
AF = mybir.ActivationFunctionType

KC = D // 128         # 8 contraction chunks for projections
CC = C // 128         # 24 key tiles
TT = T // 128         # 16 query tiles


def _emit(nc, tc, io):
    xT, ctxT, wqkvT, woT, bqkv, msk, iden, out = io

    with (
        tc.sbuf_pool(name="persist", bufs=1) as pp,
        tc.sbuf_pool(name="wqkv", bufs=1) as wp,
        tc.sbuf_pool(name="stream", bufs=1) as sp,
    ):
        qT = [pp.tile([128, T], BF16, name=f"qT{p}") for p in range(2)]
        kT = [pp.tile([128, C], BF16, name=f"kT{p}") for p in range(2)]
        v = pp.tile([128, CC * HL * DHP], BF16, name="v")
        wo = [pp.tile([128, D], BF16, name=f"wo{p}") for p in range(2)]
        msk_sb = pp.tile([65, 128], BF16, name="msk_sb")
        iden_sb = pp.tile([128, 128], BF16, name="iden_sb")
        b_sb = [pp.tile([128, 3], F32, name=f"b{p}") for p in range(2)]
        # ones columns for the softmax denominators: engine memset, not DMA
        # (strided 2-byte-packet DMAs at the head of the input queue cost
        # ~40us of dead PE time).
        ones_view = v.rearrange("a (i c) -> a i c", c=DHP)[:, :, DH:DHP]
        nc.gpsimd.memset(ones_view, 1.0)

        for p in range(2):
            nc.sync.dma_start(out=b_sb[p], in_=bqkv[ts(p, 128), :])
        bq_sb = [b_sb[p][:, 0:1] for p in range(2)]
        bk_sb = [b_sb[p][:, 1:2] for p in range(2)]
        bv_sb = [b_sb[p][:, 2:3] for p in range(2)]
        nc.sync.dma_start(out=iden_sb, in_=iden)

        w_sb = [wp.tile([128, 3 * DL], BF16, name=f"w{k}") for k in range(KC)]
        xs = [sp.tile([128, T], BF16, tag=f"x{k}", name=f"xs{k}") for k in range(KC)]
        cs = [sp.tile([128, C], BF16, tag=f"c{k}", name=f"cs{k}") for k in range(KC)]
        # DMA priority order on the (single) dynamic queue: weights and x
        # first (unblocks Q-projection ~6us in), then context, then the
        # late-phase constants (msk, wo).
        for k in range(KC):
            nc.sync.dma_start(out=w_sb[k], in_=wqkvT[ts(k, 128), :])
        for k in range(KC):
            nc.sync.dma_start(out=xs[k], in_=xT[ts(k, 128), :])
        for k in range(KC):
            nc.sync.dma_start(out=cs[k], in_=ctxT[ts(k, 128), :])
        nc.sync.dma_start(out=msk_sb, in_=msk)
        for p in range(2):
            nc.sync.dma_start(out=wo[p], in_=woT[ts(p, 128), :])

        wq = [w_sb[k][:, 0:DL] for k in range(KC)]
        wk = [w_sb[k][:, DL : 2 * DL] for k in range(KC)]
        wv = [w_sb[k][:, 2 * DL : 3 * DL] for k in range(KC)]

        # ---------------- Phase A: Q and V projections ----------------
        # k-outer loops: all output PSUM groups live at once, so matmuls
        # issue as each input k-chunk's DMA lands instead of waiting for the
        # full tensor (PE saturated from the first chunk arrival).
        with tc.psum_pool(name="qpp", bufs=8) as qpp:
            qgr = [
                qpp.tile([128, 512], F32, tag="proj", name=f"qp{p}{tq}")
                for p in range(2)
                for tq in range(4)
            ]
            for k in range(KC):
                for i in range(8):
                    p, tq = i // 4, i % 4
                    nc.tensor.matmul(
                        qgr[i],
                        lhsT=wq[k][:, ts(p, 128)],
                        rhs=xs[k][:, ts(tq, 512)],
                        start=(k == 0),
                        stop=(k == KC - 1),
                    )
            for i in range(8):
                p, tq = i // 4, i % 4
                nc.vector.tensor_scalar_add(
                    out=qT[p][:, ts(tq, 512)], in0=qgr[i], scalar1=bq_sb[p]
                )

        # V projection + transposes (K is interleaved into attention)
        with (
            tc.sbuf_pool(name="vstage", bufs=4) as vp,
            tc.psum_pool(name="pps", bufs=6) as pps,
            tc.psum_pool(name="tps", bufs=2) as tps,
        ):
            for p in range(2):
                vgr = [
                    pps.tile([128, 512], F32, tag="proj", name=f"vp{p}{cq}")
                    for cq in range(6)
                ]
                for k in range(KC):
                    for cq in range(6):
                        nc.tensor.matmul(
                            vgr[cq],
                            lhsT=wv[k][:, ts(p, 128)],
                            rhs=cs[k][:, ts(cq, 512)],
                            start=(k == 0),
                            stop=(k == KC - 1),
                        )
                for cq in range(6):
                    vst = vp.tile([128, 512], BF16, tag="vst", name=f"vs{p}{cq}")
                    nc.vector.tensor_scalar_add(out=vst, in0=vgr[cq], scalar1=bv_sb[p])
                    for cb in range(4):
                        ci = cq * 4 + cb
                        tp_ = tps.tile([128, 128], BF16, tag="tr", name=f"tr{ci}{p}")
                        nc.tensor.transpose(tp_, vst[:, ts(cb, 128)], iden_sb)
                        src_ = tp_.rearrange("a (h c) -> a h c", h=2)
                        dst = v[:, ds(ci * HL * DHP + 2 * p * DHP, 2 * DHP)].rearrange(
                            "a (h c) -> a h c", h=2
                        )[:, :, 0:DH]
                        nc.vector.tensor_copy(out=dst, in_=src_)

        # ---------------- Phases B + C (K-proj + out-proj interleaved) ----
        with (
            tc.sbuf_pool(name="uop", bufs=1) as up,
            tc.sbuf_pool(name="rsp", bufs=1) as rp,
            tc.sbuf_pool(name="obp", bufs=3) as ob,
            tc.sbuf_pool(name="expp", bufs=3) as ep,
            tc.sbuf_pool(name="rcp", bufs=2) as rc,
            tc.psum_pool(name="qkps", bufs=2) as qps,
            tc.psum_pool(name="uops", bufs=2) as ups,
        ):
            uo = [up.tile([128, T], BF16, name=f"uo{pr}") for pr in range(2)]
            rs = [rp.tile([65, T], BF16, name=f"rs{pr}") for pr in range(2)]
            for pr in range(2):
                nc.gpsimd.memset(rs[pr], 1.0)

            def kproj_block(pp_, cq):
                def emit():
                    ps = qps.tile([128, 512], F32, tag="qk", name=f"kp{pp_}{cq}")
                    for k in range(KC):
                        nc.tensor.matmul(
                            ps,
                            lhsT=wk[k][:, ts(pp_, 128)],
                            rhs=cs[k][:, ts(cq, 512)],
                            start=(k == 0),
                            stop=(k == KC - 1),
                        )
                    nc.vector.tensor_scalar_add(
                        out=kT[pp_][:, ts(cq, 512)], in0=ps, scalar1=bk_sb[pp_]
                    )

                return emit

            def c_tile(tt, on_act=False):
                # one out-projection t-tile: out[tt*128 : , :] = sum_p uo_p @ wo_p
                def emit():
                    osb = ob.tile([128, D], BF16, tag="ob", name=f"ob{tt}")
                    for dc in range(2):
                        o_ps = qps.tile([128, 512], F32, tag="qk", name=f"o{tt}_{dc}")
                        for p in range(2):
                            nc.tensor.matmul(
                                o_ps,
                                lhsT=uo[p][:, ts(tt, 128)],
                                rhs=wo[p][:, ts(dc, 512)],
                                start=(p == 0),
                                stop=(p == 1),
                            )
                        if on_act and dc == 1:
                            nc.scalar.copy(out=osb[:, ts(dc, 512)], in_=o_ps)
                        else:
                            nc.vector.tensor_copy(out=osb[:, ts(dc, 512)], in_=o_ps)
                    nc.sync.dma_start(out=out[ts(tt, 128), :], in_=osb)

                return emit

            def norm_pair(pr, th):
                # broadcast denominators [2 rows] -> [128, 1024], recip, scale uo
                bc = qps.tile([128, 1024], F32, tag="qk", name=f"bc{pr}_{th}")
                for tq in range(2):
                    nc.tensor.matmul(
                        bc[:, ts(tq, 512)],
                        lhsT=msk_sb,
                        rhs=rs[pr][:, ds(th * 1024 + tq * 512, 512)],
                        start=True,
                        stop=True,
                    )
                rcl = rc.tile([128, 1024], F32, tag="rc", name=f"rcl{pr}_{th}")
                nc.vector.reciprocal_approx_fast(rcl, bc)
                nc.vector.tensor_mul(
                    out=uo[pr][:, ds(th * 1024, 1024)],
                    in0=uo[pr][:, ds(th * 1024, 1024)],
                    in1=rcl,
                )

            def attn_pair(p, tb, fillers):
                # heads (2p, 2p+1) together: their QK matmuls go to PE row
                # tiles (0,0)/(64,0) back-to-back, executing concurrently.
                uo_ps = [
                    ups.tile([65, 1024], F32, tag="uo", name=f"up{p}{tb}_{h2}")
                    for h2 in range(2)
                ]

                def pv_step(ci, h2, ex):
                    h = 2 * p + h2
                    vsl = v[:, ds((ci * HL + h) * DHP, DHP)]
                    for tq in range(2):
                        nc.tensor.matmul(
                            uo_ps[h2][:, ts(tq, 512)],
                            lhsT=vsl,
                            rhs=ex[:, ts(tq, 512)],
                            start=(ci == 0),
                            stop=(ci == CC - 1),
                        )

                pend = deque()
                for ci in range(CC):
                    for f in fillers.get(ci, ()):
                        f()
                    exs = []
                    for h2 in range(2):
                        base = 64 * h2
                        qk = qps.tile(
                            [128, 1024], F32, tag="qk", name=f"qk{p}{tb}_{ci}_{h2}"
                        )
                        for tq in range(2):
                            nc.tensor.matmul(
                                qk[:, ts(tq, 512)],
                                lhsT=kT[p][ds(base, 64), ts(ci, 128)],
                                rhs=qT[p][ds(base, 64), ds(tb * 1024 + tq * 512, 512)],
                                start=True,
                                stop=True,
                                tile_position=(base, 0),
                            )
                        ex = ep.tile(
                            [128, 1024], BF16, tag="exp", name=f"ex{p}{tb}_{ci}_{h2}"
                        )
                        nc.scalar.activation(ex, qk, AF.Exp, scale=0.125)
                        exs.append(ex)
                    while pend:
                        pv_step(*pend.popleft())
                    for h2 in range(2):
                        pend.append((ci, h2, exs[h2]))
                while pend:
                    pv_step(*pend.popleft())
                # drain: split the two head copies across ACT and DVE (the
                # boundary is the one place ACT is idle), keeping the
                # next pair's start off this critical path.
                nc.scalar.copy(
                    out=uo[p][ds(0, 64), ds(tb * 1024, 1024)],
                    in_=uo_ps[0][0:64, :],
                )
                nc.vector.tensor_copy(
                    out=uo[p][ds(64, 64), ds(tb * 1024, 1024)],
                    in_=uo_ps[1][0:64, :],
                )
                for h2 in range(2):
                    nc.vector.tensor_copy(
                        out=rs[p][ds(64 * h2, 1), ds(tb * 1024, 1024)],
                        in_=uo_ps[h2][64:65, :],
                    )

            # Norms run as fillers inside the NEXT pair (off the boundary
            # critical path); K-proj feeds pairs 1-2 just-in-time, one cq
            # block ahead; out-projection t-tiles 0-7 hide in pairs 3-4.
            # pair 1 (p0,tb0)
            attn_pair(
                0,
                0,
                {
                    0: [kproj_block(0, 0), kproj_block(0, 1)],
                    4: [kproj_block(0, 2)],
                    8: [kproj_block(0, 3)],
                    12: [kproj_block(0, 4)],
                    16: [kproj_block(0, 5)],
                    20: [kproj_block(1, 0)],
                    22: [kproj_block(1, 1)],
                },
            )
            # pair 2 (p1,tb0)
            attn_pair(
                1,
                0,
                {
                    2: [lambda: norm_pair(0, 0)],
                    4: [kproj_block(1, 2)],
                    8: [kproj_block(1, 3)],
                    12: [kproj_block(1, 4)],
                    16: [kproj_block(1, 5)],
                },
            )
            # pair 3 (p0,tb1)
            attn_pair(
                0,
                1,
                {
                    1: [lambda: norm_pair(1, 0)],
                    4: [c_tile(0)],
                    7: [c_tile(1)],
                    10: [c_tile(2)],
                    13: [c_tile(3)],
                },
            )
            # pair 4 (p1,tb1)
            attn_pair(
                1,
                1,
                {
                    1: [lambda: norm_pair(0, 1)],
                    4: [c_tile(4)],
                    7: [c_tile(5)],
                    10: [c_tile(6)],
                    13: [c_tile(7)],
                },
            )
            norm_pair(1, 1)
            for tt in range(TT // 2, TT):
                c_tile(tt, on_act=True)()


def _build_nc():
    nc = bacc.Bacc("TRN2", target_bir_lowering=False, debug=False, num_devices=NCORES)
    xT = nc.dram_tensor("xT", [D, T], BF16, kind="ExternalInput").ap()
    ctxT = nc.dram_tensor("ctxT", [D, C], BF16, kind="ExternalInput").ap()
    wqkvT = nc.dram_tensor("wqkvT", [D, 3 * DL], BF16, kind="ExternalInput").ap()
    woT = nc.dram_tensor("woT", [DL, D], BF16, kind="ExternalInput").ap()
    bqkv = nc.dram_tensor("bqkv", [DL, 3], F32, kind="ExternalInput").ap()
    msk = nc.dram_tensor("msk", [65, 128], BF16, kind="ExternalInput").ap()
    iden = nc.dram_tensor("iden", [128, 128], BF16, kind="ExternalInput").ap()
    out = nc.dram_tensor("out", [T, D], BF16, kind="ExternalOutput").ap()
    with tile.TileContext(nc) as tc:
        _emit(nc, tc, (xT, ctxT, wqkvT, woT, bqkv, msk, iden, out))
    nc.compile()
    return nc


_NC_CACHE = None


def _get_nc():
    global _NC_CACHE
    if _NC_CACHE is None:
        _NC_CACHE = _build_nc()
    return _NC_CACHE


def _make_in_maps(inputs):
    x = np.asarray(inputs["x"], dtype=np.float32)
    context = np.asarray(inputs["context"], dtype=np.float32)
    Wq = np.asarray(inputs["Wq"], dtype=np.float32)
    Wk = np.asarray(inputs["Wk"], dtype=np.float32)
    Wv = np.asarray(inputs["Wv"], dtype=np.float32)
    Wo = np.asarray(inputs["Wo"], dtype=np.float32)
    bq = np.asarray(inputs["bq"], dtype=np.float32)
    bk = np.asarray(inputs["bk"], dtype=np.float32)
    bv = np.asarray(inputs["bv"], dtype=np.float32)

    msk = np.zeros((65, 128), _bf16)
    msk[0, :64] = 1.0
    msk[64, 64:] = 1.0
    iden = np.eye(128, dtype=_bf16)

    xTs = [np.ascontiguousarray(x[b].T).astype(_bf16) for b in range(B)]
    cTs = [np.ascontiguousarray(context[b].T).astype(_bf16) for b in range(B)]

    in_maps = []
    for core in range(NCORES):
        b, hg = core // 4, core % 4
        sl = slice(hg * DL, (hg + 1) * DL)
        in_maps.append(
            {
                "xT": xTs[b],
                "ctxT": cTs[b],
                "wqkvT": np.ascontiguousarray(
                    np.concatenate([Wq[sl].T, Wk[sl].T, Wv[sl].T], axis=1)
                ).astype(_bf16),
                "woT": np.ascontiguousarray(Wo[:, sl].T).astype(_bf16),
                "bqkv": np.ascontiguousarray(
                    np.stack([bq[sl], bk[sl], bv[sl]], axis=1)
                ),
                "msk": msk,
                "iden": iden,
            }
        )
    return in_maps


def run_spmd(inputs, trace=False):
    """Run the SPMD kernel; returns (full output [B,T,D], BassKernelResults)."""
    in_maps = _make_in_maps(inputs)
    res = run_bass_kernel_spmd(
        _get_nc(), in_maps, core_ids=list(range(NCORES)), trace=trace
    )
    bo = np.asarray(inputs["bo"], dtype=np.float32)
    y = np.zeros((B, T, D), np.float32)
    for core in range(NCORES):
        y[core // 4] += np.asarray(res.results[core]["out"], dtype=np.float32)
    y += bo.reshape(1, 1, D)
    return y, res


def kernel(**inputs):
    y, _ = run_spmd(inputs, trace=False)
    return y
